# revision 1
# baseline (speedup 1.0000x reference)
"""Graphormer encoder layer on 8 trn2 NeuronCores.

Sharding: batch (4) x query-half (2) -> 8 cores, no collectives.
Core c handles batch b=c//2, query rows [q0, q0+448) with q0=(c%2)*448.
Only the first 896 sequence positions are computed (last 128 are padded:
keys are masked out and the corresponding output rows are zeroed by the
reference, so we never touch them); padded output rows are zero-filled on
the host.

Numerics: bf16 matmuls with fp32 PSUM accumulation; softmax uses
exp(s)*exp(bias) (no max-subtraction; scores are O(1) so exp is safe);
attention row-sums come from 64 replicated ones-columns appended to V so
the normalization divisor lands on PSUM partitions 64..127 (no partition
broadcast needed).

Layout convention: "feature-major" activations X.T [features, tokens] feed
matmuls; softmax/LayerNorm run on natural [tokens, features] tiles.
"""

import sys

sys.path.insert(0, "/opt/trn_rl_repo")

import numpy as np
import ml_dtypes

import concourse.bass as bass
import concourse.tile as tile
from concourse import bacc, mybir
from concourse.bass_utils import run_bass_kernel_spmd
from concourse.masks import make_identity

BF16 = mybir.dt.bfloat16
F32 = mybir.dt.float32
AF = mybir.ActivationFunctionType
ALU = mybir.AluOpType

B, S, H, NH, F = 4, 1024, 1024, 16, 4096
HD = H // NH          # 64
PAD = 128
SV = S - PAD          # 896 valid rows
R = SV // 2           # 448 query rows per core
NKT = SV // 128       # 7 k tiles
NHC = H // 128        # 8 chunks of H
NFT = F // 128        # 32 tiles of F
EPS = 1e-5
# q tiles within the 448 rows (last one ragged)
QT = [(0, 128), (128, 128), (256, 128), (384, 64)]


def build_program():
    nc = bacc.Bacc("TRN2", target_bir_lowering=False, debug=False)

    d_xT = nc.dram_tensor("xT", [H, SV], BF16, kind="ExternalInput")
    d_xq = nc.dram_tensor("xq", [R, H], F32, kind="ExternalInput")
    d_biasT = nc.dram_tensor("biasT", [SV, R], BF16, kind="ExternalInput")
    d_qkvw = nc.dram_tensor("qkvw", [H, 3 * H], BF16, kind="ExternalInput")
    d_qkvb = nc.dram_tensor("qkvb", [3 * H, 1], F32, kind="ExternalInput")
    d_projw = nc.dram_tensor("projw", [H, H], BF16, kind="ExternalInput")
    d_w1 = nc.dram_tensor("w1", [H, F], BF16, kind="ExternalInput")
    d_b1 = nc.dram_tensor("b1", [F, 1], F32, kind="ExternalInput")
    d_w2 = nc.dram_tensor("w2", [F, H], BF16, kind="ExternalInput")
    # rows: ln1_g, ln1_b, ln2_g, ln2_b, ffn_b2
    d_lnp = nc.dram_tensor("lnp", [5, H], F32, kind="ExternalInput")
    d_out = nc.dram_tensor("out", [R, H], F32, kind="ExternalOutput")

    def bcast_row(dram_ap, offset_elems, row_len, nparts=128):
        """AP reading row_len dram elems replicated across nparts partitions."""
        base = dram_ap.ap()
        return bass.AP(
            tensor=base.tensor,
            offset=base.offset + offset_elems,
            ap=[[0, nparts], [1, row_len]],
        )

    with tile.TileContext(nc) as tc:
        with (
            tc.tile_pool(name="const", bufs=1) as const,
            tc.tile_pool(name="g3", bufs=1) as g3,  # attnT: lives C -> D
        ):
            ident = const.tile([128, 128], F32)
            make_identity(nc, ident)
            ones64 = const.tile([128, 64], BF16, tag="ones64")
            nc.vector.memset(ones64[:], 1.0)
            eps_t = const.tile([128, 1], F32, tag="eps")
            nc.vector.memset(eps_t[:], EPS)
            qkb = const.tile([128, 16], F32, tag="qkb")  # Q,K biases per m-tile
            nc.sync.dma_start(
                qkb[:],
                d_qkvb.ap()[: 16 * 128, :].rearrange("(m p) one -> p (m one)", p=128),
            )
            b1t = const.tile([128, NFT], F32, tag="b1t")
            nc.sync.dma_start(
                b1t[:], d_b1.ap().rearrange("(f p) one -> p (f one)", p=128)
            )
            attnT = g3.tile([128, NHC, R], BF16, tag="attnT")

            with tc.tile_pool(name="g2", bufs=1) as g2:  # qkv outs: B -> C
                biasT_sb = g2.tile([128, NKT, R], BF16, tag="biasT")
                nc.sync.dma_start(
                    biasT_sb[:], d_biasT.ap().rearrange("(t p) q -> p t q", p=128)
                )
                identb = g2.tile([128, 128], BF16, tag="identb")
                nc.vector.tensor_copy(identb[:], ident[:])
                qT = g2.tile([128, NHC, R], BF16, tag="qT")
                kT = g2.tile([128, NHC, SV], BF16, tag="kT")
                vnat = g2.tile([128, NKT, H], BF16, tag="vnat")

                # ---------------- Phase B: QKV ----------------
                with (
                    tc.tile_pool(name="qkvw", bufs=1) as wpool,
                    tc.tile_pool(name="xT", bufs=1) as xpool,
                    tc.tile_pool(name="psB", bufs=4, space="PSUM") as psB,
                ):
                    vb_bc = wpool.tile([128, H], F32, tag="vb")
                    nc.sync.dma_start(vb_bc[:], bcast_row(d_qkvb, 2 * H, H))
                    qkvw_sb = wpool.tile([128, NHC, 3 * H], BF16, tag="qkvw")
                    xT_sb = xpool.tile([128, NHC, SV], BF16, tag="xT")
                    for kc in range(NHC):
                        nc.sync.dma_start(
                            xT_sb[:, kc, :], d_xT.ap()[kc * 128 : (kc + 1) * 128, :]
                        )
                        nc.sync.dma_start(
                            qkvw_sb[:, kc, :],
                            d_qkvw.ap()[kc * 128 : (kc + 1) * 128, :],
                        )

                    # host rolls x rows so this core's own 448 q rows are
                    # always xT cols 0:448 (bias key axis rolled to match)
                    for m in range(NHC):  # Q^T feature tiles
                        ps = psB.tile([128, 512], F32, tag="psB")
                        for kc in range(NHC):
                            nc.tensor.matmul(
                                ps[:, :R],
                                qkvw_sb[:, kc, m * 128 : (m + 1) * 128],
                                xT_sb[:, kc, 0:R],
                                start=(kc == 0),
                                stop=(kc == NHC - 1),
                            )
                        nc.scalar.activation(
                            qT[:, m, :], ps[:, :R], AF.Identity,
                            bias=qkb[:, m : m + 1],
                        )
                    for m in range(NHC):  # K^T feature tiles
                        for n in range(2):
                            ps = psB.tile([128, 512], F32, tag="psB")
                            for kc in range(NHC):
                                nc.tensor.matmul(
                                    ps[:, :R],
                                    qkvw_sb[:, kc, H + m * 128 : H + (m + 1) * 128],
                                    xT_sb[:, kc, n * R : (n + 1) * R],
                                    start=(kc == 0),
                                    stop=(kc == NHC - 1),
                                )
                            nc.scalar.activation(
                                kT[:, m, n * R : (n + 1) * R],
                                ps[:, :R],
                                AF.Identity,
                                bias=qkb[:, 8 + m : 9 + m],
                            )
                    for t in range(NKT):  # V natural [k rows, v features]
                        for n in range(2):
                            ps = psB.tile([128, 512], F32, tag="psB")
                            for kc in range(NHC):
                                nc.tensor.matmul(
                                    ps[:],
                                    xT_sb[:, kc, t * 128 : (t + 1) * 128],
                                    qkvw_sb[
                                        :, kc,
                                        2 * H + n * 512 : 2 * H + (n + 1) * 512,
                                    ],
                                    start=(kc == 0),
                                    stop=(kc == NHC - 1),
                                )
                            nc.vector.tensor_tensor(
                                out=vnat[:, t, n * 512 : (n + 1) * 512],
                                in0=ps[:],
                                in1=vb_bc[:, n * 512 : (n + 1) * 512],
                                op=ALU.add,
                            )

                # ---------------- Phase C: attention ----------------
                with (
                    tc.tile_pool(name="epool", bufs=2) as epool,
                    tc.tile_pool(name="spool", bufs=3, space="PSUM") as spool,
                    tc.tile_pool(name="opool", bufs=2, space="PSUM") as opool,
                    tc.tile_pool(name="rpool", bufs=3) as rpool,
                ):
                    for m in range(NH // 2):  # head pairs -> 128-part tiles
                        Es = []
                        for j in range(2):
                            po = 64 * j
                            E = epool.tile([128, NKT, R], BF16, tag=f"E{j}",
                                           name=f"E{j}")
                            Es.append(E)
                            for t in range(NKT):
                                ps = spool.tile([128, R], F32, tag="sc")
                                nc.tensor.matmul(
                                    ps[:],
                                    kT[po : po + 64, m, t * 128 : (t + 1) * 128],
                                    qT[po : po + 64, m, :],
                                    start=True,
                                    stop=False,
                                )
                                nc.tensor.matmul(
                                    ps[:],
                                    identb[:],
                                    biasT_sb[:, t, :],
                                    start=False,
                                    stop=True,
                                )
                                nc.scalar.activation(E[:, t, :], ps[:], AF.Exp)
                        psv = opool.tile([128, R], F32, tag="av")
                        pss = opool.tile([128, R], F32, tag="sm")
                        for j in range(2):
                            h = 2 * m + j
                            po = 64 * j
                            for t in range(NKT):
                                nc.tensor.matmul(
                                    psv[po : po + 64, :],
                                    vnat[:, t, h * 64 : (h + 1) * 64],
                                    Es[j][:, t, :],
                                    start=(t == 0),
                                    stop=(t == NKT - 1),
                                )
                            for t in range(NKT):
                                nc.tensor.matmul(
                                    pss[po : po + 64, :],
                                    ones64[:],
                                    Es[j][:, t, :],
                                    start=(t == 0),
                                    stop=(t == NKT - 1),
                                )
                        rec = rpool.tile([128, R], F32, tag="rec")
                        nc.vector.reciprocal(rec[:], pss[:])
                        nc.vector.tensor_tensor(
                            out=attnT[:, m, :], in0=psv[:], in1=rec[:], op=ALU.mult
                        )

            # ---------------- Phase D: proj + LN1 + transpose ----------------
            with tc.tile_pool(name="g5", bufs=1) as g5:  # y, yT live D -> E
                y_sb = g5.tile([128, 4, H], F32, tag="y")
                yT = g5.tile([128, NHC, R], BF16, tag="yT")
                with (
                    tc.tile_pool(name="projw", bufs=1) as pwpool,
                    tc.tile_pool(name="ppool", bufs=2, space="PSUM") as ppool,
                    tc.tile_pool(name="tpool", bufs=2, space="PSUM") as tpool,
                    tc.tile_pool(name="lpool", bufs=2) as lpool,
                ):
                    ln1g = lpool.tile([128, H], F32, tag="ln1g")
                    nc.sync.dma_start(ln1g[:], bcast_row(d_lnp, 0, H))
                    ln1b = lpool.tile([128, H], F32, tag="ln1b")
                    nc.sync.dma_start(ln1b[:], bcast_row(d_lnp, H, H))
                    xq_sb = lpool.tile([128, 4, H], F32, tag="xq")
                    for i, (o, sz) in enumerate(QT):
                        nc.sync.dma_start(xq_sb[:sz, i, :], d_xq.ap()[o : o + sz, :])
                    projw_sb = pwpool.tile([128, NHC, H], BF16, tag="projw")
                    for kc in range(NHC):
                        nc.sync.dma_start(
                            projw_sb[:, kc, :],
                            d_projw.ap()[kc * 128 : (kc + 1) * 128, :],
                        )
                    for i, (o, sz) in enumerate(QT):
                        ps = ppool.tile([128, H], F32, tag="proj")
                        for n in range(2):
                            for kc in range(NHC):
                                nc.tensor.matmul(
                                    ps[:sz, n * 512 : (n + 1) * 512],
                                    attnT[:, kc, o : o + sz],
                                    projw_sb[:, kc, n * 512 : (n + 1) * 512],
                                    start=(kc == 0),
                                    stop=(kc == NHC - 1),
                                )
                        # residual (xq already includes proj_b) + LN1
                        r = lpool.tile([128, H], F32, tag="r")
                        nc.vector.tensor_tensor(
                            out=r[:sz], in0=ps[:sz], in1=xq_sb[:sz, i, :], op=ALU.add
                        )
                        self_ln(nc, lpool, r, sz, ln1g, ln1b, y_sb[:, i, :], eps_t)
                        # transpose y tile -> yT
                        for kc in range(NHC):
                            pt = tpool.tile([128, 128], F32, tag="tr")
                            nc.tensor.transpose(
                                pt[:, :sz],
                                y_sb[:sz, i, kc * 128 : (kc + 1) * 128],
                                ident[:sz, :sz],
                            )
                            nc.scalar.activation(
                                yT[:, kc, o : o + sz], pt[:, :sz], AF.Copy
                            )

                # ---------------- Phase E: FFN ----------------
                with tc.tile_pool(name="g6", bufs=1) as g6:  # hT: E1 -> E2
                    hT = g6.tile([128, NFT, R], BF16, tag="hT")
                    with (
                        tc.tile_pool(name="w1pool", bufs=1) as w1pool,
                        tc.tile_pool(name="hpool", bufs=2, space="PSUM") as hpool,
                    ):
                        w1_sb = w1pool.tile([128, NHC, F], BF16, tag="w1")
                        for kc in range(NHC):
                            nc.sync.dma_start(
                                w1_sb[:, kc, :],
                                d_w1.ap()[kc * 128 : (kc + 1) * 128, :],
                            )
                        for f in range(NFT):
                            ps = hpool.tile([128, R], F32, tag="h")
                            for kc in range(NHC):
                                nc.tensor.matmul(
                                    ps[:],
                                    w1_sb[:, kc, f * 128 : (f + 1) * 128],
                                    yT[:, kc, :],
                                    start=(kc == 0),
                                    stop=(kc == NHC - 1),
                                )
                            nc.scalar.activation(
                                hT[:, f, :], ps[:], AF.Gelu, bias=b1t[:, f : f + 1]
                            )

                    with (
                        tc.tile_pool(name="w2pool", bufs=6) as w2pool,
                        tc.tile_pool(name="zpool", bufs=2, space="PSUM") as zpool,
                        tc.tile_pool(name="l2pool", bufs=2) as l2pool,
                    ):
                        ln2g = l2pool.tile([128, H], F32, tag="ln2g")
                        nc.sync.dma_start(ln2g[:], bcast_row(d_lnp, 2 * H, H))
                        ln2b = l2pool.tile([128, H], F32, tag="ln2b")
                        nc.sync.dma_start(ln2b[:], bcast_row(d_lnp, 3 * H, H))
                        fb2 = l2pool.tile([128, H], F32, tag="fb2")
                        nc.sync.dma_start(fb2[:], bcast_row(d_lnp, 4 * H, H))
                        out_sb = l2pool.tile([128, 4, H], F32, tag="out")
                        for g in range(2):  # 2 groups of 2 q-tiles: w2 is
                            # streamed twice; LN2 of group 0 overlaps group 1
                            zts = {}
                            for i in (2 * g, 2 * g + 1):
                                zts[i] = zpool.tile(
                                    [128, H], F32, tag=f"z{i % 2}", name=f"z{i % 2}"
                                )
                            for fc in range(NFT):
                                w2c = w2pool.tile([128, H], BF16, tag="w2c")
                                nc.sync.dma_start(
                                    w2c[:], d_w2.ap()[fc * 128 : (fc + 1) * 128, :]
                                )
                                for i in (2 * g, 2 * g + 1):
                                    o, sz = QT[i]
                                    for n in range(2):
                                        nc.tensor.matmul(
                                            zts[i][:sz, n * 512 : (n + 1) * 512],
                                            hT[:, fc, o : o + sz],
                                            w2c[:, n * 512 : (n + 1) * 512],
                                            start=(fc == 0),
                                            stop=(fc == NFT - 1),
                                        )
                            for i in (2 * g, 2 * g + 1):
                                o, sz = QT[i]
                                zt = zts[i]
                                r = l2pool.tile([128, H], F32, tag="r2")
                                nc.vector.tensor_tensor(
                                    out=r[:sz], in0=zt[:sz], in1=y_sb[:sz, i, :],
                                    op=ALU.add,
                                )
                                nc.vector.tensor_tensor(
                                    out=r[:sz], in0=r[:sz], in1=fb2[:sz, :],
                                    op=ALU.add,
                                )
                                self_ln(
                                    nc, l2pool, r, sz, ln2g, ln2b,
                                    out_sb[:, i, :], eps_t,
                                )
                                nc.sync.dma_start(
                                    d_out.ap()[o : o + sz, :], out_sb[:sz, i, :]
                                )

    nc.compile()
    return nc


def self_ln(nc, pool, r, sz, g_bc, b_bc, out_ap, eps_t):
    """LayerNorm over the free dim of r[:sz] (width H), writes out_ap[:sz]."""
    nm = pool.tile([128, 1], F32, tag="nm")
    nc.vector.tensor_reduce(
        out=nm[:sz], in_=r[:sz], axis=mybir.AxisListType.X, op=ALU.add
    )
    nc.vector.tensor_scalar_mul(nm[:sz], nm[:sz], -1.0 / H)
    sq = pool.tile([128, H], F32, tag="sq")
    nc.scalar.activation(sq[:sz], r[:sz], AF.Square, bias=nm[:sz])
    var = pool.tile([128, 1], F32, tag="var")
    nc.vector.tensor_reduce(
        out=var[:sz], in_=sq[:sz], axis=mybir.AxisListType.X, op=ALU.add
    )
    sd = pool.tile([128, 1], F32, tag="sd")
    nc.scalar.activation(sd[:sz], var[:sz], AF.Sqrt, scale=1.0 / H, bias=eps_t[:sz])
    rstd = pool.tile([128, 1], F32, tag="rstd")
    nc.vector.reciprocal(rstd[:sz], sd[:sz])
    t = pool.tile([128, H], F32, tag="lt")
    nc.vector.tensor_scalar(
        out=t[:sz],
        in0=r[:sz],
        scalar1=nm[:sz],
        scalar2=rstd[:sz],
        op0=ALU.add,
        op1=ALU.mult,
    )
    nc.vector.tensor_tensor(out=t[:sz], in0=t[:sz], in1=g_bc[:sz, :], op=ALU.mult)
    nc.vector.tensor_tensor(out=out_ap[:sz], in0=t[:sz], in1=b_bc[:sz, :], op=ALU.add)


_NC = None


def _get_nc():
    global _NC
    if _NC is None:
        _NC = build_program()
    return _NC


def _prep_inputs(x, attn_bias, key_padding_mask, qkv_w, qkv_b, proj_w, proj_b,
                 ln1_g, ln1_b, ln2_g, ln2_b, ffn_w1, ffn_b1, ffn_w2, ffn_b2):
    bf = ml_dtypes.bfloat16
    scale = HD ** -0.5
    qkv_ws = np.array(qkv_w, dtype=np.float32, copy=True)
    qkv_ws[:, :H] *= scale
    qkv_bs = np.array(qkv_b, dtype=np.float32, copy=True)
    qkv_bs[:H] *= scale
    shared = {
        "qkvw": qkv_ws.astype(bf),
        "qkvb": qkv_bs.reshape(3 * H, 1).astype(np.float32),
        "projw": np.asarray(proj_w).astype(bf),
        "w1": np.asarray(ffn_w1).astype(bf),
        "b1": np.asarray(ffn_b1).reshape(F, 1).astype(np.float32),
        "w2": np.asarray(ffn_w2).astype(bf),
        "lnp": np.stack(
            [ln1_g, ln1_b, ln2_g, ln2_b, ffn_b2]
        ).astype(np.float32),
    }
    in_maps = []
    x = np.asarray(x, dtype=np.float32)
    attn_bias = np.asarray(attn_bias, dtype=np.float32)
    proj_b = np.asarray(proj_b, dtype=np.float32)
    for c in range(8):
        b, half = c // 2, c % 2
        q0 = half * R
        # roll x columns so this core's own q rows occupy cols 0:448 of xT
        xv = x[b, :SV, :]          # [896, H]
        rolled = np.roll(xv, -q0, axis=0) if q0 else xv
        m = dict(shared)
        m["xT"] = np.ascontiguousarray(rolled.T).astype(bf)
        m["xq"] = (x[b, q0 : q0 + R, :] + proj_b[None, :]).astype(np.float32)
        # key axis must follow the same roll applied to xT's rows
        bT = np.ascontiguousarray(attn_bias[b, q0 : q0 + R, :SV].T)
        if q0:
            bT = np.roll(bT, -q0, axis=0)
        m["biasT"] = bT.astype(bf)
        in_maps.append(m)
    return in_maps


def _assemble(results, dtype):
    out = np.zeros((B, S, H), dtype=np.float32)
    for c in range(8):
        b, half = c // 2, c % 2
        q0 = half * R
        out[b, q0 : q0 + R, :] = results[c]["out"]
    return out.astype(dtype)


def kernel(**inputs):
    nc = _get_nc()
    in_maps = _prep_inputs(**inputs)
    res = run_bass_kernel_spmd(nc, in_maps, list(range(8)))
    return _assemble(res.results, np.asarray(inputs["x"]).dtype)


def kernel_profiled(inputs, tmpdir=None):
    nc = _get_nc()
    in_maps = _prep_inputs(**inputs)
    res = run_bass_kernel_spmd(
        nc, in_maps, list(range(8)), trace=True, tmpdir=tmpdir
    )
    return _assemble(res.results, np.float32), res



# revision 12
# speedup vs baseline: 1.3278x; 1.3278x over previous
"""Graphormer encoder layer on 8 trn2 NeuronCores.

Sharding: batch (4) x query-half (2) -> 8 cores, no collectives.
Core c handles batch b=c//2, query rows [q0, q0+448) with q0=(c%2)*448.
Only the first 896 sequence positions are computed (last 128 are padded:
keys are masked out, and the reference zeroes those output rows).

Design (cost-model driven):
- Everything is "feature-major": activations live as X.T [features, tokens]
  so LayerNorm reductions become PE matmuls against a ones-vector and
  per-feature affine params are per-partition scalars.
- softmax: exp(s + bias) = exp(s) * exp(bias); exp(bias) is precomputed on
  the host, so the PE never touches the bias. Row sums come from 64
  ones-columns interleaved into the V stationary operand, so one matmul per
  (head, key-tile) yields both attn@V (partitions 0:64) and the softmax
  divisor (partitions 64:128); a single DVE divide normalizes.
- QKV runs kc-outer so the first matmul only needs the first weight chunk;
  attention per head-pair is interleaved into QKV so the scalar-engine exp
  stream overlaps PE work.
- FFN: w1 resident (prefetched from program start into fresh SBUF), w2
  streamed once per token-half; FFN2 accumulates all 8 output chunks of a
  token-half in 4 PSUM banks (2 accumulators per bank). LayerNorm of half A
  overlaps FFN2 matmuls of half B.
"""

import sys
from contextlib import ExitStack

sys.path.insert(0, "/opt/trn_rl_repo")

import numpy as np
import ml_dtypes

import concourse.bass as bass
import concourse.tile as tile
from concourse import bacc, mybir
from concourse.bass_utils import run_bass_kernel_spmd

BF16 = mybir.dt.bfloat16
F32 = mybir.dt.float32
AF = mybir.ActivationFunctionType
ALU = mybir.AluOpType

B, S, H, NH, F = 4, 1024, 1024, 16, 4096
HD = H // NH          # 64
PAD = 128
SV = S - PAD          # 896 valid rows
R = SV // 2           # 448 query rows per core
NKT = SV // 128       # 7 key tiles
NHC = H // 128        # 8 chunks of H
NFT = F // 128        # 32 tiles of F
EPS = 1e-5
TH = R // 2           # 224-token half


def build_program():
    nc = bacc.Bacc("TRN2", target_bir_lowering=False, debug=False)

    d_xT = nc.dram_tensor("xT", [H, SV], BF16, kind="ExternalInput")
    d_qw = nc.dram_tensor("qw", [H, H], BF16, kind="ExternalInput")
    d_kw = nc.dram_tensor("kw", [H, H], BF16, kind="ExternalInput")
    d_vw = nc.dram_tensor("vw", [H, H], BF16, kind="ExternalInput")
    d_expBT = nc.dram_tensor("expBT", [SV, R], BF16, kind="ExternalInput")
    d_projw = nc.dram_tensor("projw", [H, H], BF16, kind="ExternalInput")
    d_xqT = nc.dram_tensor("xqT", [H, R], F32, kind="ExternalInput")
    d_w1 = nc.dram_tensor("w1", [H, F], BF16, kind="ExternalInput")
    d_w2 = nc.dram_tensor("w2", [F, H], BF16, kind="ExternalInput")
    d_qkb = nc.dram_tensor("qkb", [128, 16], F32, kind="ExternalInput")
    d_b1t = nc.dram_tensor("b1t", [128, NFT], F32, kind="ExternalInput")
    # lnc rows: 0 ln1_g, 1 ln1_b, 2 ln2_g, 3 ln2_b, 4 ffn_b2  ([128, 5, 8])
    d_lnc = nc.dram_tensor("lnc", [128, 5 * NHC], F32, kind="ExternalInput")
    d_out = nc.dram_tensor("out", [H, R], F32, kind="ExternalOutput")

    with tile.TileContext(nc) as tc, ExitStack() as ctx:
        # ---------- long-lived pools ----------
        const = ctx.enter_context(tc.tile_pool(name="const", bufs=1))
        eps_t = const.tile([128, 1], F32, tag="eps")
        nc.vector.memset(eps_t[:], EPS)
        ones_bf = const.tile([128, 128], BF16, tag="ones")
        nc.vector.memset(ones_bf[:], 1.0)
        qkb = const.tile([128, 16], F32, tag="qkb")
        b1t = const.tile([128, NFT], F32, tag="b1t")
        lnc = const.tile([128, 5, NHC], F32, tag="lnc")
        nc.gpsimd.dma_start(qkb[:], d_qkb.ap())
        nc.gpsimd.dma_start(b1t[:], d_b1t.ap())
        nc.gpsimd.dma_start(lnc[:], d_lnc.ap().rearrange("p (r c) -> p r c", r=5))

        # right-side long-lived: ln-output chain tiles + proj inputs
        pfm = ctx.enter_context(tc.tile_pool(name="pfm", bufs=1, side="right"))
        yT = pfm.tile([128, NHC, R], BF16, tag="yT")      # LN1 out (FFN1 in)
        p1 = ctx.enter_context(tc.tile_pool(name="p1", bufs=1, side="right"))
        attnT = p1.tile([128, NHC, R], BF16, tag="attnT")
        projw_sb = p1.tile([128, NHC, H], BF16, tag="projw")
        xqT_sb = p1.tile([128, NHC, R], F32, tag="xqT")
        for r in range(2):
            nc.gpsimd.dma_start(
                projw_sb[:, 4 * r : 4 * r + 4, :],
                d_projw.ap()[r * 512 : (r + 1) * 512, :].rearrange(
                    "(c p) h -> p c h", p=128
                ),
            )
        nc.gpsimd.dma_start(
            xqT_sb[:], d_xqT.ap().rearrange("(c p) q -> p c q", p=128)
        )

        # ---------- phase B + C ----------
        with (
            tc.tile_pool(name="gqkv", bufs=1) as gqkv,
            tc.tile_pool(name="epool", bufs=3) as epool,
            tc.tile_pool(name="erpool", bufs=2) as erpool,
            tc.tile_pool(name="scp", bufs=2, space="PSUM") as scp,
            tc.tile_pool(name="avp", bufs=2, space="PSUM") as avp,
        ):
            qT = gqkv.tile([128, NHC, R], BF16, tag="qT")
            kT = gqkv.tile([128, NHC, SV], BF16, tag="kT")
            vno = gqkv.tile([128, NKT, NH, 128], BF16, tag="vno")
            expBT_sb = gqkv.tile([128, NKT, R], BF16, tag="expBT")
            # interleaved ones columns for the softmax row sums (Pool engine,
            # one-time; must land before the first attn@V matmul)
            nc.gpsimd.memset(vno[:, :, :, 64:128], 1.0)
            nc.sync.dma_start(
                expBT_sb[:], d_expBT.ap().rearrange("(t p) q -> p t q", p=128)
            )

            def c_scores(m, j):
                """scores + exp + bias-mult for head 2m+j -> E tile."""
                po = 64 * j
                E = epool.tile([128, NKT, R], BF16, tag="E", name="E")
                for t in range(NKT):
                    sc = scp.tile([128, R], F32, tag="sc", name="sc")
                    nc.tensor.matmul(
                        sc[:],
                        kT[po : po + 64, m, t * 128 : (t + 1) * 128],
                        qT[po : po + 64, m, :],
                        start=True,
                        stop=True,
                    )
                    er = erpool.tile([128, R], BF16, tag="er", name="er")
                    nc.scalar.activation(er[:], sc[:], AF.Exp)
                    nc.vector.tensor_tensor(
                        out=E[:, t, :], in0=er[:], in1=expBT_sb[:, t, :],
                        op=ALU.mult,
                    )
                return E

            def c_av(m, j, E):
                """attn@V + rowsum via interleaved ones; divide -> attnT."""
                po = 64 * j
                h = 2 * m + j
                psv = avp.tile([128, R], F32, tag="av", name="psv")
                for t in range(NKT):
                    nc.tensor.matmul(
                        psv[:],
                        vno[:, t, h, :],
                        E[:, t, :],
                        start=(t == 0),
                        stop=(t == NKT - 1),
                    )
                rec = erpool.tile([128, R], F32, tag="rec", name="rec")
                nc.vector.reciprocal(rec[64:128, :], psv[64:128, :])
                nc.vector.tensor_tensor(
                    out=attnT[po : po + 64, m, :],
                    in0=psv[0:64, :],
                    in1=rec[64:128, :],
                    op=ALU.mult,
                )

            def c_head(m):
                E0 = c_scores(m, 0)
                E1 = c_scores(m, 1)
                c_av(m, 0, E0)
                c_av(m, 1, E1)

            with (
                tc.tile_pool(name="gB", bufs=1) as gB,
                tc.tile_pool(name="psB", bufs=4, space="PSUM") as psB,
            ):
                xT_sb = gB.tile([128, NHC, SV], BF16, tag="xT")
                qw_sb = gB.tile([128, NHC, H], BF16, tag="qw")
                kw_sb = gB.tile([128, NHC, H], BF16, tag="kw")
                vw_sb = gB.tile([128, NHC, H], BF16, tag="vw")
                for kc in range(NHC):
                    nc.sync.dma_start(
                        qw_sb[:, kc, :], d_qw.ap()[kc * 128 : (kc + 1) * 128, :]
                    )
                    nc.sync.dma_start(
                        xT_sb[:, kc, :], d_xT.ap()[kc * 128 : (kc + 1) * 128, :]
                    )
                for kc in range(NHC):
                    nc.sync.dma_start(
                        kw_sb[:, kc, :], d_kw.ap()[kc * 128 : (kc + 1) * 128, :]
                    )
                for kc in range(NHC):
                    nc.sync.dma_start(
                        vw_sb[:, kc, :], d_vw.ap()[kc * 128 : (kc + 1) * 128, :]
                    )

                def q_pass(ms):
                    ps = {m: psB.tile([128, 512], F32, tag="psB", name=f"psB{m%4}") for m in ms}
                    for kc in range(NHC):
                        for m in ms:
                            nc.tensor.matmul(
                                ps[m][:, :R],
                                qw_sb[:, kc, m * 128 : (m + 1) * 128],
                                xT_sb[:, kc, 0:R],
                                start=(kc == 0),
                                stop=(kc == NHC - 1),
                            )
                    for m in ms:
                        nc.scalar.activation(
                            qT[:, m, :], ps[m][:, :R], AF.Identity,
                            bias=qkb[:, m : m + 1],
                        )

                def k_pass(ms, n):
                    ps = {m: psB.tile([128, 512], F32, tag="psB", name=f"psB{m%4}") for m in ms}
                    for kc in range(NHC):
                        for m in ms:
                            nc.tensor.matmul(
                                ps[m][:, :R],
                                kw_sb[:, kc, m * 128 : (m + 1) * 128],
                                xT_sb[:, kc, n * R : (n + 1) * R],
                                start=(kc == 0),
                                stop=(kc == NHC - 1),
                            )
                    for m in ms:
                        nc.scalar.activation(
                            kT[:, m, n * R : (n + 1) * R], ps[m][:, :R],
                            AF.Identity, bias=qkb[:, 8 + m : 9 + m],
                        )

                def v_pass(ts, n):
                    ps = {t: psB.tile([128, 512], F32, tag="psB", name=f"psV{t%4}") for t in ts}
                    for kc in range(NHC):
                        for t in ts:
                            nc.tensor.matmul(
                                ps[t][:],
                                xT_sb[:, kc, t * 128 : (t + 1) * 128],
                                vw_sb[:, kc, n * 512 : (n + 1) * 512],
                                start=(kc == 0),
                                stop=(kc == NHC - 1),
                            )
                    for t in ts:
                        nc.vector.tensor_copy(
                            vno[:, t, 8 * n : 8 * n + 8, 0:64], ps[t][:]
                        )

                q_pass([0, 1, 2, 3])
                k_pass([0, 1, 2, 3], 0)
                k_pass([0, 1, 2, 3], 1)
                E00 = c_scores(0, 0)
                E01 = c_scores(0, 1)
                v_pass([0, 1, 2, 3], 0)
                v_pass([4, 5, 6], 0)
                c_av(0, 0, E00)
                c_av(0, 1, E01)
                c_head(1)
                q_pass([4, 5, 6, 7])
                c_head(2)
                k_pass([4, 5, 6, 7], 0)
                c_head(3)
                k_pass([4, 5, 6, 7], 1)
                v_pass([0, 1, 2, 3], 1)
                v_pass([4, 5, 6], 1)
                c_head(4)

            # w1 rotating range stream (4 x 512-feature ranges in flight)
            w1p = ctx.enter_context(
                tc.tile_pool(name="w1p", bufs=4, side="right")
            )
            w1t = {}
            for r in range(NHC):
                w1t[r] = w1p.tile([128, NHC, 512], BF16, tag="w1", name="w1t")
                nc.sync.dma_start(
                    w1t[r][:],
                    d_w1.ap()[:, r * 512 : (r + 1) * 512].rearrange(
                        "(c p) f -> p c f", p=128
                    ),
                )
            c_head(5)
            c_head(6)
            c_head(7)

        # ---------- phase D: proj + LN1 (feature-major) ----------
        hp = ctx.enter_context(tc.tile_pool(name="hp", bufs=1, side="right"))
        hT = hp.tile([128, NFT, R], BF16, tag="hT")
        # [r | r^2] pairs per token-half; written by LN1 residual, reused by LN2
        rsqA = hp.tile([128, NHC, 2, TH], BF16, tag="rsqA")
        rsqB = hp.tile([128, NHC, 2, TH], BF16, tag="rsqB")
        # resident w2 on the (empty) left side, streamed via the gpsimd queue
        w2r_pool = ctx.enter_context(tc.tile_pool(name="w2r", bufs=1))
        w2r = w2r_pool.tile([128, NFT, H], BF16, tag="w2r")
        for r in range(NHC):
            nc.gpsimd.dma_start(
                w2r[:, 4 * r : 4 * r + 4, :],
                d_w2.ap()[r * 512 : (r + 1) * 512, :].rearrange(
                    "(f p) h -> p f h", p=128
                ),
            )

        def ln_stats(rsq, s1):
            """One matmul per chunk over the [r | r^2] pair: s1[:, 0:TH] gets
            sum(r), s1[:, TH:2*TH] gets sum(r^2). Single accumulation group
            per PSUM bank (interleaved groups lose their first chunk: start
            clears the whole bank's has_written)."""
            for c in range(NHC):
                nc.tensor.matmul(
                    s1[:, 0 : 2 * TH], ones_bf[:], rsq[:, c, :, :],
                    start=(c == 0), stop=(c == NHC - 1),
                )

        def ln_norm(lpool, s1, rsq, grow, brow, out_tile, post=None):
            """rstd chain + per-chunk normalize.
            out_tile(c) = (rsq[:,c,0,:] - mu) * rstd * g + b."""
            nmu = lpool.tile([128, TH], F32, tag="nmu", name="nmu")
            nc.vector.tensor_scalar_mul(nmu[:], s1[:, 0:TH], -1.0 / H)
            musq = lpool.tile([128, TH], F32, tag="musq", name="musq")
            nc.vector.tensor_tensor(out=musq[:], in0=nmu[:], in1=nmu[:], op=ALU.mult)
            var = lpool.tile([128, TH], F32, tag="var", name="var")
            nc.vector.scalar_tensor_tensor(
                out=var[:], in0=s1[:, TH : 2 * TH], scalar=1.0 / H,
                in1=musq[:], op0=ALU.mult, op1=ALU.subtract,
            )
            sd = lpool.tile([128, TH], F32, tag="sd", name="sd")
            nc.scalar.activation(sd[:], var[:], AF.Sqrt, bias=eps_t[:, 0:1])
            rstd = lpool.tile([128, TH], F32, tag="rstd", name="rstd")
            nc.vector.reciprocal(rstd[:], sd[:])
            nmr = lpool.tile([128, TH], F32, tag="nmr", name="nmr")
            nc.vector.tensor_tensor(out=nmr[:], in0=nmu[:], in1=rstd[:], op=ALU.mult)
            for c in range(NHC):
                t1 = lpool.tile([128, TH], F32, tag="t1", name="t1")
                nc.vector.tensor_tensor(
                    out=t1[:], in0=rsq[:, c, 0, :], in1=rstd[:], op=ALU.mult
                )
                t2 = lpool.tile([128, TH], F32, tag="t2", name="t2")
                nc.vector.tensor_tensor(out=t2[:], in0=t1[:], in1=nmr[:], op=ALU.add)
                nc.scalar.activation(
                    out_tile(c), t2[:], AF.Identity,
                    scale=lnc[:, grow, c : c + 1], bias=lnc[:, brow, c : c + 1],
                )
                if post is not None:
                    post(c)

        with (
            tc.tile_pool(name="ppp", bufs=2, space="PSUM") as ppp,
            tc.tile_pool(name="s1p", bufs=2, space="PSUM") as s1p,
            tc.tile_pool(name="lp", bufs=2) as lp,
        ):
            s1h = {}
            rsqh = {0: rsqA, 1: rsqB}
            for ha in range(2):
                sl = slice(ha * TH, (ha + 1) * TH)
                for c in range(NHC):
                    pp = ppp.tile([128, TH], F32, tag="pp", name="pp")
                    for kc in range(NHC):
                        nc.tensor.matmul(
                            pp[:],
                            projw_sb[:, kc, c * 128 : (c + 1) * 128],
                            attnT[:, kc, sl],
                            start=(kc == 0),
                            stop=(kc == NHC - 1),
                        )
                    nc.vector.tensor_tensor(
                        out=rsqh[ha][:, c, 0, :], in0=pp[:], in1=xqT_sb[:, c, sl],
                        op=ALU.add,
                    )
                    nc.scalar.activation(
                        rsqh[ha][:, c, 1, :], rsqh[ha][:, c, 0, :], AF.Square
                    )
                s1h[ha] = s1p.tile([128, 512], F32, tag="s1", name="s1")
            for ha in range(2):
                ln_stats(rsqh[ha], s1h[ha])
            for ha in range(2):
                sl = slice(ha * TH, (ha + 1) * TH)
                ln_norm(
                    lp, s1h[ha], rsqh[ha], 0, 1,
                    lambda c, _sl=sl: yT[:, c, _sl],
                )

        # ---------- phase E: FFN ----------
        with (
            tc.tile_pool(name="fpp", bufs=2, space="PSUM") as fpp,
            tc.tile_pool(name="zpp", bufs=4, space="PSUM") as zpp,
            tc.tile_pool(name="s2p", bufs=2, space="PSUM") as s2p,
            tc.tile_pool(name="l2p", bufs=2) as l2p,
            tc.tile_pool(name="orp", bufs=2, side="right") as orp,
        ):
            # FFN1: f-outer so each w1 range streams once; halves per range so
            # the first range only needs half A of yT
            for r in range(NHC):
                for ha in range(2):
                    sl = slice(ha * TH, (ha + 1) * TH)
                    for fr in range(4):
                        f = 4 * r + fr
                        ph = fpp.tile([128, TH], F32, tag="ph", name="ph")
                        for kc in range(NHC):
                            nc.tensor.matmul(
                                ph[:],
                                w1t[r][:, kc, fr * 128 : (fr + 1) * 128],
                                yT[:, kc, sl],
                                start=(kc == 0),
                                stop=(kc == NHC - 1),
                            )
                        nc.scalar.activation(
                            hT[:, f, sl], ph[:], AF.Gelu, bias=b1t[:, f : f + 1]
                        )

            def ffn2_pass(ha, grp):
                """4 output chunks of one token-half, each in its own bank."""
                sl = slice(ha * TH, (ha + 1) * TH)
                zps = [
                    zpp.tile([128, 512], F32, tag="z", name=f"z{i}")
                    for i in range(4)
                ]
                for fc in range(NFT):
                    for i in range(4):
                        c = 4 * grp + i
                        nc.tensor.matmul(
                            zps[i][:, 0:TH],
                            w2r[:, fc, c * 128 : (c + 1) * 128],
                            hT[:, fc, sl],
                            start=(fc == 0),
                            stop=(fc == NFT - 1),
                        )
                return zps

            def ffn2_post(ha, grp, zps, rsq):
                sl = slice(ha * TH, (ha + 1) * TH)
                for i in range(4):
                    c = 4 * grp + i
                    nc.vector.scalar_tensor_tensor(
                        out=rsq[:, c, 0, :], in0=zps[i][:, 0:TH],
                        scalar=lnc[:, 4, c : c + 1], in1=yT[:, c, sl],
                        op0=ALU.add, op1=ALU.add,
                    )
                    nc.scalar.activation(rsq[:, c, 1, :], rsq[:, c, 0, :], AF.Square)

            def ln2_finish(ha, s2, rsq):
                sl = slice(ha * TH, (ha + 1) * TH)
                tiles = {}

                def emit(c):
                    ot = orp.tile([128, TH], F32, tag="ot", name="ot")
                    tiles[c] = ot
                    return ot[:]

                def post(c):
                    nc.gpsimd.dma_start(
                        d_out.ap().rearrange("(c p) q -> p c q", p=128)[:, c, sl],
                        tiles[c][:],
                    )

                ln_norm(l2p, s2, rsq, 2, 3, emit, post=post)

            s2A = s2p.tile([128, 512], F32, tag="s2", name="s2A")
            zA0 = ffn2_pass(0, 0)
            ffn2_post(0, 0, zA0, rsqA)
            zA1 = ffn2_pass(0, 1)
            ffn2_post(0, 1, zA1, rsqA)
            ln_stats(rsqA, s2A)
            zB0 = ffn2_pass(1, 0)
            ffn2_post(1, 0, zB0, rsqB)
            ln2_finish(0, s2A, rsqA)
            s2B = s2p.tile([128, 512], F32, tag="s2", name="s2B")
            zB1 = ffn2_pass(1, 1)
            ffn2_post(1, 1, zB1, rsqB)
            ln_stats(rsqB, s2B)
            ln2_finish(1, s2B, rsqB)

    nc.compile()
    return nc


_NC = None


def _get_nc():
    global _NC
    if _NC is None:
        _NC = build_program()
    return _NC


def _prep_inputs(x, attn_bias, key_padding_mask, qkv_w, qkv_b, proj_w, proj_b,
                 ln1_g, ln1_b, ln2_g, ln2_b, ffn_w1, ffn_b1, ffn_w2, ffn_b2):
    bf = ml_dtypes.bfloat16
    scale = HD ** -0.5
    qkv_w = np.asarray(qkv_w, dtype=np.float32)
    qkv_b = np.asarray(qkv_b, dtype=np.float32)
    qw = (qkv_w[:, :H] * scale).astype(bf)
    kw = qkv_w[:, H : 2 * H].astype(bf)
    vw = qkv_w[:, 2 * H :].astype(bf)
    bq = qkv_b[:H] * scale
    bk = qkv_b[H : 2 * H]
    bv = qkv_b[2 * H :]
    proj_w = np.asarray(proj_w, dtype=np.float32)
    proj_b = np.asarray(proj_b, dtype=np.float32)
    # residual base: x rows + proj_b + bv @ proj_w  (attn weights sum to 1)
    cvec = proj_b + bv @ proj_w

    # per-chunk [128, c] layouts for per-partition scalars
    def chunked(v):
        return np.ascontiguousarray(
            np.asarray(v, np.float32).reshape(-1, 128).T
        )  # [128, nchunk]

    qkb = np.concatenate([chunked(bq), chunked(bk)], axis=1).astype(np.float32)
    b1t = chunked(ffn_b1).astype(np.float32)
    lnc = np.concatenate(
        [chunked(ln1_g), chunked(ln1_b), chunked(ln2_g), chunked(ln2_b),
         chunked(ffn_b2)],
        axis=1,
    ).astype(np.float32)

    shared = {
        "qw": qw, "kw": kw, "vw": vw,
        "projw": proj_w.astype(bf),
        "w1": np.asarray(ffn_w1).astype(bf),
        "w2": np.asarray(ffn_w2).astype(bf),
        "qkb": qkb, "b1t": b1t, "lnc": lnc,
    }
    x = np.asarray(x, dtype=np.float32)
    attn_bias = np.asarray(attn_bias, dtype=np.float32)
    in_maps = []
    for c in range(8):
        b, half = c // 2, c % 2
        q0 = half * R
        xv = x[b, :SV, :]          # [896, H]
        rolled = np.roll(xv, -q0, axis=0) if q0 else xv
        m = dict(shared)
        m["xT"] = np.ascontiguousarray(rolled.T).astype(bf)
        m["xqT"] = np.ascontiguousarray(
            (x[b, q0 : q0 + R, :] + cvec[None, :]).T
        ).astype(np.float32)
        bT = np.ascontiguousarray(attn_bias[b, q0 : q0 + R, :SV].T)
        if q0:
            bT = np.roll(bT, -q0, axis=0)
        m["expBT"] = np.exp(bT).astype(bf)
        in_maps.append(m)
    return in_maps


def _assemble(results, dtype):
    out = np.zeros((B, S, H), dtype=np.float32)
    for c in range(8):
        b, half = c // 2, c % 2
        q0 = half * R
        out[b, q0 : q0 + R, :] = results[c]["out"].T
    return out.astype(dtype)


def kernel(**inputs):
    nc = _get_nc()
    in_maps = _prep_inputs(**inputs)
    res = run_bass_kernel_spmd(nc, in_maps, list(range(8)))
    return _assemble(res.results, np.asarray(inputs["x"]).dtype)


def kernel_profiled(inputs, tmpdir=None):
    nc = _get_nc()
    in_maps = _prep_inputs(**inputs)
    res = run_bass_kernel_spmd(
        nc, in_maps, list(range(8)), trace=True, tmpdir=tmpdir
    )
    return _assemble(res.results, np.float32), res
